# revision 1
# baseline (speedup 1.0000x reference)
"""KNN anomaly-scoring kernel for Trainium2 (Bass/Tile), 8 NeuronCores.

Model: for each of B=8 images with a [768, 32, 32] embedding grid, compute the
mean Euclidean distance to the 3 nearest neighbors in a 20000x768 memory bank
per spatial location, then bilinear-upsample the 32x32 score map to 512x512.

Sharding: data-parallel over batch. Core b handles image b (1024 queries) with
a full bank replica. No collectives.

Per-core device program (fp8 DoubleRow matmuls + 3-engine top-k scan):
  - Ranking key on the TensorEngine in fp8e4 DoubleRow mode (2 k-tiles of 128
    per instruction, 0.5 cycles/output column):
      r[q, n] = 2q.b - (b2[n] - 768)
    via 3 DoubleRow matmuls over E=768 plus one k=1 DoubleRow "aug" matmul
    whose two pairs carry (b2-768) as a two-term fp8 expansion (coarse +
    residual), keeping b2 nearly exact. d2 = (q2 + 768) - r with q2 applied
    at the end as a per-partition activation bias.
  - Bank padded 20000 -> 20480 (pad entries get r = -448, never selected),
    10 groups of 2048 columns, 8 query tiles -> 80 chunks. Each chunk's PSUM
    is two 2-bank tiles (psA: sub-chunks 0-1, psB: 2-3) so the ACT copy of
    psA starts mid-chunk and the PSUM ring never gates the PE.
  - Scan per chunk, balanced across the three PSUM/SBUF engines (~1.75us
    each, matching the PE's 1.71us of matmul):
      ACT: copy cols [0:1024] (psA) and [1024:1648] (psB) to SBUF as bf16
      DVE: top-8 of psB[624:1024] straight from PSUM (f32), and -- deferred
           by one chunk so it never waits on a fresh copy -- a bf16
           tensor_tensor(max) fold chain 1648->824->412->206 (2x DVE mode)
           plus top-8 of the folded 206. 16 candidates per chunk.
  - Final per query tile: top-8 of the 160 candidates, then ACT
    Sqrt(r * (-1/9) + (q2+768)/9) with accum_out -> mean 3-NN distance / 1.
  - Scores [128, 8] -> S[32, 32] by in-place DVE 32x32 block transposes;
    upsample O = R S R^T as one K=128 matmul against a host-replicated
    zero-padded R^T (rtp4) followed by 4 matmuls with u stationary, all
    bf16 moving operands (R entries k/16 are exact in bf16).

TimelineSim-modeled device time ~165.5us vs ~512us for the f32r baseline.
"""

import numpy as np

B, E, HL, WL = 8, 768, 32, 32
N_BANK = 20000
Q = HL * WL            # 1024 queries per image
QT = Q // 128          # 8 query tiles
KC = E // 128          # 6 contraction k-tiles of 128
KK = KC // 2           # 3 DoubleRow pairs
OUT = 512
NPAD = 20480
NG = 10                # bank column groups
GW = NPAD // NG        # 2048 columns per group
SC = GW // 512         # 4 matmul sub-chunks per group (one PSUM bank each)
WA = 1648              # ACT copy width per chunk (DVE folds 8->1 in bf16)
WTAIL = GW - WA        # DVE direct-psum max8 width (400)
NCAND = NG * 16        # 160 candidate slots per query

_CACHE = {}


def _build_nc():
    import concourse.bass as bass
    import concourse.bacc as bacc
    import concourse.mybir as mybir
    from concourse.tile import TileContext

    f32 = mybir.dt.float32
    f8 = mybir.dt.float8e4
    bf16 = mybir.dt.bfloat16
    DR = mybir.MatmulPerfMode.DoubleRow

    nc = bacc.Bacc("TRN2", target_bir_lowering=False, debug=False)

    qk8_d = nc.dram_tensor("qk8", [128, KC, Q], f8, kind="ExternalInput")
    qb_d = nc.dram_tensor("qb", [128, QT], f32, kind="ExternalInput")
    bank8_d = nc.dram_tensor("bank8", [NG, KC, 128, GW], f8, kind="ExternalInput")
    aug_d = nc.dram_tensor("aug", [NG, 2, GW], f8, kind="ExternalInput")
    rt_d = nc.dram_tensor("rt", [32, OUT], bf16, kind="ExternalInput")
    rtp4_d = nc.dram_tensor("rtp4", [128, OUT], bf16, kind="ExternalInput")
    out_d = nc.dram_tensor("out", [OUT, OUT], f32, kind="ExternalOutput")

    with TileContext(nc) as tc:
        with (
            tc.tile_pool(name="qpool", bufs=1) as qpool,
            tc.tile_pool(name="bpool", bufs=2) as bpool,
            tc.tile_pool(name="cpool", bufs=1) as cpool,
            tc.tile_pool(name="spool", bufs=3) as spool,
            tc.tile_pool(name="fpool", bufs=2) as fpool,
            tc.tile_pool(name="ppool", bufs=2, space=bass.MemorySpace.PSUM) as ppool,
        ):
            # ---- persistent tiles; DMAs ordered so chunk 0 starts early ----
            qk_all = qpool.tile([128, KC, Q], f8, tag="qk")
            qb_sb = qpool.tile([128, QT], f32, tag="qb")
            rt_sb = qpool.tile([32, OUT], bf16, tag="rt")
            rtp4_sb = qpool.tile([128, OUT], bf16, tag="rtp4")
            neg1_sb = qpool.tile([1, 2, 128], f8, tag="neg1")
            nc.vector.memset(neg1_sb[:], -1.0)

            cand = [
                cpool.tile([128, NCAND], f32, tag=f"cand{t}", name=f"cand{t}")
                for t in range(QT)
            ]
            scores_sb = cpool.tile([128, 32], f32, tag="scores")
            nc.vector.memset(scores_sb[:], 0.0)

            def emit_final(t):
                fin8 = fpool.tile([128, 8], f32, tag="fin8")
                nc.vector.max(fin8[:], cand[t][:])
                d3 = fpool.tile([128, 3], f32, tag="d3")
                nc.scalar.activation(
                    d3[:], fin8[:, 0:3], mybir.ActivationFunctionType.Sqrt,
                    scale=-1.0 / 9.0, bias=qb_sb[:, t:t + 1],
                    accum_out=scores_sb[:, t:t + 1],
                )

            def emit_folds(cp, g, t):
                # bf16 fold chain 1648 -> 824 -> 412 -> 206, then top-8.
                f1 = spool.tile([128, WA // 2], bf16, tag="f1")
                nc.vector.tensor_tensor(
                    f1[:], cp[:, 0:WA // 2], cp[:, WA // 2:WA],
                    op=mybir.AluOpType.max,
                )
                f2 = spool.tile([128, WA // 4], bf16, tag="f2")
                nc.vector.tensor_tensor(
                    f2[:], f1[:, 0:WA // 4], f1[:, WA // 4:WA // 2],
                    op=mybir.AluOpType.max,
                )
                f3 = spool.tile([128, WA // 8], bf16, tag="f3")
                nc.vector.tensor_tensor(
                    f3[:], f2[:, 0:WA // 8], f2[:, WA // 8:WA // 4],
                    op=mybir.AluOpType.max,
                )
                nc.vector.max(cand[t][:, g * 16:g * 16 + 8], f3[:])
                if g == NG - 1:
                    emit_final(t)

            # ---- ranking key + per-chunk top-16 (1-chunk fold pipeline) ----
            prev = None
            for g in range(NG):
                bk = [bpool.tile([128, 2, GW], f8, tag=f"bank{kk}",
                                 name=f"bk{kk}")
                      for kk in range(KK)]
                au = bpool.tile([1, 2, GW], f8, tag="aug")
                if g == 0:
                    # interleave so the first sub-chunk's inputs arrive first
                    for j in range(2):
                        nc.sync.dma_start(bk[0][:, j, :], bank8_d[g, j])
                    nc.sync.dma_start(qk_all[:, 0:2, :], qk8_d[:, 0:2, :])
                    nc.sync.dma_start(au[:], aug_d[g])
                    for kk in range(1, KK):
                        nc.sync.dma_start(qk_all[:, 2 * kk:2 * kk + 2, :],
                                          qk8_d[:, 2 * kk:2 * kk + 2, :])
                        for j in range(2):
                            nc.sync.dma_start(bk[kk][:, j, :],
                                              bank8_d[g, 2 * kk + j])
                    nc.sync.dma_start(qb_sb[:, 0:QT], qb_d[:])
                    nc.sync.dma_start(rt_sb[:], rt_d[:])
                    nc.sync.dma_start(rtp4_sb[:], rtp4_d[:])
                else:
                    for kk in range(KK):
                        for j in range(2):
                            nc.sync.dma_start(bk[kk][:, j, :],
                                              bank8_d[g, 2 * kk + j])
                    nc.sync.dma_start(au[:], aug_d[g])
                for t in range(QT):
                    # Two 2-bank PSUM tiles per chunk: psA (sub-chunks 0-1)
                    # drains via copy_a mid-chunk, decoupling the PSUM ring
                    # from the PE critical path.
                    psA = ppool.tile([128, GW // 2], f32, tag="megaA")
                    psB = ppool.tile([128, GW // 2], f32, tag="megaB")
                    cp = spool.tile([128, WA], bf16, tag="cp")
                    for c in range(SC):
                        ps = psA if c < 2 else psB
                        dst = ps[:, (c % 2) * 512:(c % 2 + 1) * 512]
                        for kk in range(KK):
                            nc.tensor.matmul(
                                dst,
                                qk_all[:, 2 * kk:2 * kk + 2,
                                       t * 128:(t + 1) * 128],
                                bk[kk][:, :, c * 512:(c + 1) * 512],
                                start=(kk == 0), stop=False, perf_mode=DR,
                            )
                        nc.tensor.matmul(
                            dst,
                            neg1_sb[:],
                            au[:, :, c * 512:(c + 1) * 512],
                            start=False, stop=True, perf_mode=DR,
                        )
                        if c == 1:
                            nc.scalar.copy(cp[:, 0:1024], psA[:])
                    nc.scalar.copy(cp[:, 1024:WA], psB[:, 0:WA - 1024])
                    # Fold chain for the PREVIOUS chunk first (its inputs are
                    # ready before this chunk's PE-done), then the PSUM tail
                    # top-8 -- avoids head-of-line blocking in DVE's 4-deep
                    # wait queue.
                    if prev is not None:
                        emit_folds(*prev)
                    nc.vector.max(cand[t][:, g * 16 + 8:g * 16 + 16],
                                  psB[:, WA - 1024:])
                    prev = (cp, g, t)
            emit_folds(*prev)

            # ---- scores -> S via in-place DVE block transposes ----
            # Query q = t*128 + p, (h, w) = (q//32, q%32): score for S[h, w]
            # sits at scores_sb[32j + w, t] with h = 4t + j. Transposing each
            # 32x32 block in place puts S[4t+j, w] at partition 32j + t, col
            # w. Stage 1 contracts the scattered S rows with rtp4 (host-
            # replicated R^T rows at matching partitions) via 4 accumulating
            # K=8 matmuls.
            scores_bf = fpool.tile([128, 32], bf16, tag="scores_bf")
            nc.vector.tensor_copy(scores_bf[:], scores_sb[:])
            trp = fpool.tile([128, 32], bf16, tag="trp")
            for j in range(4):
                nc.vector.transpose(trp[32 * j:32 * (j + 1), :],
                                    scores_bf[32 * j:32 * (j + 1), :])

            # ---- bilinear upsample: out = R @ S @ R^T (bf16 inputs) ----
            # Single K=128 contraction: partitions 32j+t (t<8) carry S rows;
            # all other partitions are zeros (memset) x zero rtp4 rows.
            psu = ppool.tile([128, GW // 2], f32, tag="megaA")
            nc.tensor.matmul(psu[:32, :OUT], trp[:], rtp4_sb[:],
                             start=True, stop=True)
            u_sb = fpool.tile([32, OUT], bf16, tag="u")
            nc.vector.tensor_copy(u_sb[:], psu[:32, :OUT])
            for i in range(4):
                po = ppool.tile([128, GW // 2], f32, tag="megaB")
                nc.tensor.matmul(po[:, :OUT], u_sb[:, i * 128:(i + 1) * 128],
                                 rt_sb[:], start=True, stop=True)
                osb = fpool.tile([128, OUT], f32, tag=f"osb{i % 2}")
                if i % 2 == 0:
                    nc.vector.tensor_copy(osb[:], po[:, :OUT])
                else:
                    nc.scalar.copy(osb[:], po[:, :OUT])
                nc.sync.dma_start(out_d[i * 128:(i + 1) * 128, :], osb[:])

    nc.compile()
    return nc


def _resize_matrix(n_in: int, n_out: int) -> np.ndarray:
    """Bilinear (half-pixel, edge-clamped) interpolation matrix [n_out, n_in].
    Matches jax.image.resize(method='bilinear') for upsampling."""
    R = np.zeros((n_out, n_in), dtype=np.float64)
    scale = n_in / n_out
    for i in range(n_out):
        src = (i + 0.5) * scale - 0.5
        a0 = int(np.floor(src))
        w = src - a0
        a0c = min(max(a0, 0), n_in - 1)
        a1c = min(max(a0 + 1, 0), n_in - 1)
        R[i, a0c] += 1.0 - w
        R[i, a1c] += w
    return R.astype(np.float32)


def _prep_inputs(embeddings: np.ndarray, bank: np.ndarray):
    """Host-side layout prep. Returns per-core input maps."""
    import ml_dtypes
    f = np.float32
    f8 = ml_dtypes.float8_e4m3fn
    emb = np.asarray(embeddings, dtype=f)
    bank = np.asarray(bank, dtype=f)

    # queries: [B, E, HL, WL] -> qT [B, E, Q] (E-major for the stationary side)
    qT = emb.reshape(B, E, Q)
    q2 = np.einsum("beq,beq->bq", qT, qT)               # [B, Q]
    qk8_all = np.ascontiguousarray(
        (2.0 * qT).astype(f8).reshape(B, KC, 128, Q).transpose(0, 2, 1, 3)
    )                                                   # [B, 128, KC, Q]
    qb_all = ((q2 + 768.0) / 9.0).reshape(B, QT, 128).transpose(0, 2, 1)

    bankP = np.zeros((NPAD, E), dtype=f)
    bankP[:N_BANK] = bank
    bank8 = np.ascontiguousarray(
        bankP.T.reshape(KC, 128, NG, GW).transpose(2, 0, 1, 3).astype(f8)
    )                                                   # [NG, KC, 128, GW]
    b2c = np.full(NPAD, np.nan, dtype=f)
    b2c[:N_BANK] = np.einsum("ne,ne->n", bank, bank) - 768.0
    c0 = b2c.astype(f8)
    c1 = (b2c - c0.astype(f)).astype(f8)
    c0[N_BANK:] = f8(224.0)
    c1[N_BANK:] = f8(224.0)
    aug = np.ascontiguousarray(
        np.stack([c0, c1], axis=0).reshape(2, NG, GW).transpose(1, 0, 2)
    )                                                   # [NG, 2, GW]

    bh = ml_dtypes.bfloat16
    rt = np.ascontiguousarray(_resize_matrix(HL, OUT).T.astype(bh))  # [32, 512]
    rtp4 = np.zeros((128, OUT), dtype=bh)
    for j in range(4):
        for t in range(8):
            rtp4[32 * j + t] = rt[4 * t + j]

    in_maps = [
        {
            "qk8": np.ascontiguousarray(qk8_all[b]),
            "qb": np.ascontiguousarray(qb_all[b].astype(f)),
            "bank8": bank8,
            "aug": aug,
            "rt": rt,
            "rtp4": rtp4,
        }
        for b in range(B)
    ]
    return in_maps


def kernel(embeddings, bank, out_size, _trace=False, _trace_kwargs=None):
    from concourse import bass_utils

    assert int(out_size) == OUT
    if "nc" not in _CACHE:
        _CACHE["nc"] = _build_nc()
    nc = _CACHE["nc"]

    in_maps = _prep_inputs(np.asarray(embeddings), np.asarray(bank))
    res = bass_utils.run_bass_kernel_spmd(
        nc, in_maps, core_ids=list(range(B)), trace=_trace,
        **(_trace_kwargs or {}),
    )
    _CACHE["last_results"] = res
    out = np.stack([res.results[b]["out"] for b in range(B)])
    return out.reshape(B, 1, OUT, OUT).astype(np.float32)



# revision 32
# speedup vs baseline: 1.0598x; 1.0598x over previous
"""KNN anomaly-scoring kernel for Trainium2 (Bass/Tile), 8 NeuronCores.

Model: for each of B=8 images with a [768, 32, 32] embedding grid, compute the
mean Euclidean distance to the 3 nearest neighbors in a 20000x768 memory bank
per spatial location, then bilinear-upsample the 32x32 score map to 512x512.

Sharding: data-parallel over batch. Core b handles image b (1024 queries) with
a full bank replica. No collectives.

Per-core device program (fp8 DoubleRow matmuls + 3-engine scan):
  - Ranking key r[q, n] = 2q.b' - (b2[n] - 768) on the TensorEngine in fp8e4
    DoubleRow mode, where b' is the bank restricted to dims 0..766 and the
    b2 term rides as contraction row 767: query row 767 = 16, bank row 767 =
    -(b2-768)/16. Exactly 3 DoubleRow matmuls of k=256 per 512 output
    columns -- no separate bias matmul (vs 4 in the old kernel, a 25% PE
    saving). d2 = (q2 + 768) - r with q2 applied at the end as a per-
    partition activation bias (the dropped dim-767 cross term and the fp8
    b2 row add noise well inside the error budget; sim rel-err ~5e-4).
  - Bank padded 20000 -> 20480 (pad entries get r = -7168, never selected),
    10 groups of 2048 columns, 8 query tiles -> 80 chunks. One 4-bank PSUM
    tile [128, 2048] per chunk, filled by 4x3 DoubleRow matmuls (one PSUM
    bank per matmul output).
  - Scan per chunk, spread over ACT/DVE/Pool so no engine exceeds the PE's
    1280 ns of matmul work by much (per-chunk busy: DVE ~1334 critical,
    Pool ~1302, ACT ~1251, PE 1280):
      ACT:  ca  = bf16 copy of ps[0:1280]              (1251 ns)
      DVE:  top-8 Max straight off ps[1280:2048] -> 8-slab   (925 ns)
      Pool: p1 = max(ca.lo, ca.hi)         -> [640]     (985 ns)
      DVE:  d2 = max(p1.lo, p1.hi)         -> [320]     (226 ns, 2x)
      Pool: p2 = max(d2.lo, d2.hi)         -> [160]     (317 ns)
      DVE:  d3 -> [80], d4 -> 40-slab                   (102 + 81 ns)
    Each cross-engine stage is deferred by one chunk so the strict-FIFO
    engine queues never head-of-line block.
  - Final per query tile: top-8 of the 480 candidates, then ACT
    Sqrt(r * (-1/9) + (q2+768)/9) with accum_out -> mean 3-NN distance.
  - Scores [128, 8] -> S[32, 32] by in-place DVE 32x32 block transposes;
    upsample O = R S R^T as one K=128 matmul against a host-replicated
    zero-padded R^T (rtp4) followed by 4 matmuls with u stationary, all
    bf16 moving operands (R entries k/16 are exact in bf16).
"""

import numpy as np

B, E, HL, WL = 8, 768, 32, 32
N_BANK = 20000
Q = HL * WL            # 1024 queries per image
QT = Q // 128          # 8 query tiles
KC = E // 128          # 6 contraction k-tiles of 128
KK = KC // 2           # 3 DoubleRow pairs
OUT = 512
NPAD = 20480
NG = 10                # bank column groups
GW = NPAD // NG        # 2048 columns per group
WA = 1024              # ACT copy width per chunk (psA)
WD = GW - WA           # DVE direct-psum top-8 width (psB, 1024)
SLAB = 72              # candidate slots per chunk (8 direct + 64 folded)
NCAND = NG * SLAB      # 480 candidate slots per query
C_AUG = 16.0           # query-side constant for the merged b2 row

_CACHE = {}


def _build_nc():
    import concourse.bass as bass
    import concourse.bacc as bacc
    import concourse.mybir as mybir
    from concourse.tile import TileContext

    f32 = mybir.dt.float32
    f8 = mybir.dt.float8e4
    bf16 = mybir.dt.bfloat16
    DR = mybir.MatmulPerfMode.DoubleRow
    MAX = mybir.AluOpType.max

    nc = bacc.Bacc("TRN2", target_bir_lowering=False, debug=False)

    qk8_d = nc.dram_tensor("qk8", [128, KC, Q], f8, kind="ExternalInput")
    qb_d = nc.dram_tensor("qb", [128, QT], f32, kind="ExternalInput")
    bank8_d = nc.dram_tensor("bank8", [NG, 128, KC, GW], f8, kind="ExternalInput")
    rt_d = nc.dram_tensor("rt", [32, OUT], bf16, kind="ExternalInput")
    rtp4_d = nc.dram_tensor("rtp4", [128, OUT], bf16, kind="ExternalInput")
    out_d = nc.dram_tensor("out", [OUT, OUT], f32, kind="ExternalOutput")

    with TileContext(nc) as tc:
        with (
            tc.tile_pool(name="qpool", bufs=1) as qpool,
            tc.tile_pool(name="bpool", bufs=2) as bpool,
            tc.tile_pool(name="cpool", bufs=1) as cpool,
            tc.tile_pool(name="spool", bufs=3) as spool,
            tc.tile_pool(name="fpool", bufs=2) as fpool,
            tc.tile_pool(name="ppool", bufs=1, space=bass.MemorySpace.PSUM) as ppool,
        ):
            # ---- persistent tiles; DMAs ordered so chunk 0 starts early ----
            qk_all = qpool.tile([128, KC, Q], f8, tag="qk")
            qb_sb = qpool.tile([128, QT], f32, tag="qb")
            rt_sb = qpool.tile([32, OUT], bf16, tag="rt")
            rtp4_sb = qpool.tile([128, OUT], bf16, tag="rtp4")

            cand = [
                cpool.tile([128, NCAND], bf16, tag=f"cand{t}", name=f"cand{t}")
                for t in range(QT)
            ]
            scores_sb = cpool.tile([128, 32], f32, tag="scores")
            nc.vector.memset(scores_sb[:], 0.0)

            # candidate tiles start at -30000 so slots unwritten by a chunk
            # type (the direct-8 slots of X chunks) never win the final top-k
            for t in range(QT):
                nc.vector.memset(cand[t][:], -30000.0)

            def emit_final(t):
                finp = fpool.tile([128, NCAND // 2], bf16, tag="finp")
                nc.vector.tensor_tensor(finp[:], cand[t][:, 0:NCAND // 2],
                                        cand[t][:, NCAND // 2:NCAND], op=MAX)
                fin8 = fpool.tile([128, 8], f32, tag="fin8")
                nc.vector.max(fin8[:], finp[:])
                d3 = fpool.tile([128, 3], f32, tag="d3")
                nc.scalar.activation(
                    d3[:], fin8[:, 0:3], mybir.ActivationFunctionType.Sqrt,
                    scale=-1.0 / 9.0, bias=qb_sb[:, t:t + 1],
                    accum_out=scores_sb[:, t:t + 1],
                )

            # The fold funnel runs entirely on DVE (the real toolchain only
            # supports Memset/DMA on the Pool engine). To balance ACT and DVE,
            # chunks alternate between two types (6:5 over an 11-chunk cycle):
            #   X: ACT copies BOTH psum banks (ca+cb, 2076 ns); DVE folds the
            #      2048 bf16 cols 32:1 (L0 593 + 740 = 1333 ns).
            #   Y: ACT copies psA only (1038 ns); DVE top-8 Maxes psB straight
            #      from PSUM (1192 ns, exact) + folds ca 16:1 (740 ns).
            # Steady-state busy: ACT ~1604, DVE ~1605, PE 1286.
            # All stage inputs are >= 1 chunk old at emission so the strict-
            # FIFO engine queues never head-of-line block; stage tiles are
            # explicit mod-NR rings with single producers and consumers.
            NR = 4
            ca_r = [spool.tile([128, WA], bf16, tag=f"ca{j}", name=f"ca{j}")
                    for j in range(NR)]
            cb_r = [spool.tile([128, WD], bf16, tag=f"cb{j}", name=f"cb{j}")
                    for j in range(NR)]
            m_r = [spool.tile([128, 1024], bf16, tag=f"m{j}", name=f"m{j}")
                   for j in range(NR)]
            p1_r = [spool.tile([128, 512], bf16, tag=f"p1{j}", name=f"p1{j}")
                    for j in range(NR)]
            d2_r = [spool.tile([128, 256], bf16, tag=f"d2{j}", name=f"d2{j}")
                    for j in range(NR)]
            p2_r = [spool.tile([128, 128], bf16, tag=f"p2{j}", name=f"p2{j}")
                    for j in range(NR)]

            def is_x(i):
                return i % 11 in (0, 2, 4, 6, 8, 10)

            def st_s0(i, g, t):
                # X chunks: merge the two bank copies -> [1024]
                nc.vector.tensor_tensor(m_r[i % NR][:], ca_r[i % NR][:],
                                        cb_r[i % NR][:], op=MAX)
                return (m_r[i % NR], i, g, t)

            def st_s1(buf, i, g, t):
                nc.vector.tensor_tensor(p1_r[i % NR][:], buf[:, 0:512],
                                        buf[:, 512:1024], op=MAX)
                return (i, g, t)

            def st_s2(i, g, t):
                nc.vector.tensor_tensor(d2_r[i % NR][:], p1_r[i % NR][:, 0:256],
                                        p1_r[i % NR][:, 256:512], op=MAX)
                return (i, g, t)

            def st_s3(i, g, t):
                nc.vector.tensor_tensor(p2_r[i % NR][:], d2_r[i % NR][:, 0:128],
                                        d2_r[i % NR][:, 128:256], op=MAX)
                return (i, g, t)

            def st_s4(i, g, t):
                nc.vector.tensor_tensor(
                    cand[t][:, g * SLAB + 8:(g + 1) * SLAB],
                    p2_r[i % NR][:, 0:64], p2_r[i % NR][:, 64:128], op=MAX)
                if g == NG - 1:
                    emit_final(t)

            q_mx, q_s0, q_s1, q_s2, q_s3, q_s4 = [], [], [], [], [], []

            def pump(drain=False):
                th = 1 if drain else 2
                if len(q_mx) >= 1:
                    ps_o, g_o, t_o = q_mx.pop(0)
                    nc.vector.max(cand[t_o][:, g_o * SLAB:g_o * SLAB + 8],
                                  ps_o[:])
                if len(q_s0) >= 1:
                    q_s1.append(st_s0(*q_s0.pop(0)))
                if len(q_s1) >= th:
                    q_s2.append(st_s1(*q_s1.pop(0)))
                if len(q_s2) >= th:
                    q_s3.append(st_s2(*q_s2.pop(0)))
                if len(q_s3) >= th:
                    q_s4.append(st_s3(*q_s3.pop(0)))
                if len(q_s4) >= th:
                    st_s4(*q_s4.pop(0))

            for g in range(NG):
                bk_all = bpool.tile([128, KC, GW], f8, tag="bank",
                                    name="bk_all")
                bk = [bk_all[:, 2 * kk:2 * kk + 2, :] for kk in range(KK)]
                if g == 0:
                    # ramp: tiny first transfers so chunk (0,0)'s first
                    # sub-chunk can start ASAP, spread over three DGE queues
                    # so descriptor generation pipelines
                    nc.sync.dma_start(qk_all[:, 0:6, 0:128],
                                      qk8_d[:, 0:6, 0:128])
                    nc.scalar.dma_start(bk_all[:, 0:2, 0:512],
                                        bank8_d[g, :, 0:2, 0:512])
                    nc.gpsimd.dma_start(bk_all[:, 2:4, 0:512],
                                        bank8_d[g, :, 2:4, 0:512])
                    nc.gpsimd.dma_start(bk_all[:, 4:6, 0:512],
                                        bank8_d[g, :, 4:6, 0:512])
                    nc.sync.dma_start(bk_all[:, 0:2, 512:GW],
                                      bank8_d[g, :, 0:2, 512:GW])
                    nc.scalar.dma_start(qk_all[:, 0:6, 128:Q],
                                        qk8_d[:, 0:6, 128:Q])
                    nc.gpsimd.dma_start(bk_all[:, 2:4, 512:GW],
                                        bank8_d[g, :, 2:4, 512:GW])
                    nc.gpsimd.dma_start(bk_all[:, 4:6, 512:GW],
                                        bank8_d[g, :, 4:6, 512:GW])
                    nc.sync.dma_start(qb_sb[:, 0:QT], qb_d[:])
                    nc.sync.dma_start(rt_sb[:], rt_d[:])
                    nc.sync.dma_start(rtp4_sb[:], rtp4_d[:])
                else:
                    nc.sync.dma_start(bk_all[:], bank8_d[g])
                for t in range(QT):
                    i_ch = g * QT + t
                    # Separate bank-aligned PSUM tiles so psA's reader (ACT)
                    # and psB's reader (DVE Max) never serialize: two readers
                    # of one tile execute strictly in sequence (RAR tile
                    # ordering), which would put ca+Max on the chunk critical
                    # path and stall the PE.
                    psA = ppool.tile([128, WA], f32, tag=f"psA{i_ch % 2}",
                                     name="psA")
                    psB = ppool.tile([128, WD], f32, tag=f"psB{i_ch % 2}",
                                     name="psB")
                    for ci, dst in ((0, psA[:, 0:512]), (1, psA[:, 512:WA]),
                                    (2, psB[:, 0:512]), (3, psB[:, 512:WD])):
                        for kk in range(KK):
                            nc.tensor.matmul(
                                dst,
                                qk_all[:, 2 * kk:2 * kk + 2,
                                       t * 128:(t + 1) * 128],
                                bk[kk][:, :, ci * 512:(ci + 1) * 512],
                                start=(kk == 0), stop=(kk == KK - 1),
                                perf_mode=DR,
                            )
                    # Deferred stages first: their inputs are chunks old, so
                    # they issue immediately and never clog the strict-FIFO
                    # engine queues ahead of the fresh chunk's PSUM readers.
                    pump()
                    nc.scalar.copy(ca_r[i_ch % NR][:], psA[:])
                    if is_x(i_ch):
                        nc.scalar.copy(cb_r[i_ch % NR][:], psB[:])
                        q_s0.append((i_ch, g, t))
                    else:
                        q_mx.append((psB, g, t))
                        q_s1.append((ca_r[i_ch % NR], i_ch, g, t))
            while q_mx or q_s0 or q_s1 or q_s2 or q_s3 or q_s4:
                pump(drain=True)

            # ---- scores -> S via in-place DVE block transposes ----
            # Query q = t*128 + p, (h, w) = (q//32, q%32): score for S[h, w]
            # sits at scores_sb[32j + w, t] with h = 4t + j. Transposing each
            # 32x32 block in place puts S[4t+j, w] at partition 32j + t, col
            # w. Stage 1 contracts the scattered S rows with rtp4 (host-
            # replicated R^T rows at matching partitions).
            scores_bf = fpool.tile([128, 32], bf16, tag="scores_bf")
            nc.vector.tensor_copy(scores_bf[:], scores_sb[:])
            trp = fpool.tile([128, 32], bf16, tag="trp")
            for j in range(4):
                nc.vector.transpose(trp[32 * j:32 * (j + 1), :],
                                    scores_bf[32 * j:32 * (j + 1), :])

            # ---- bilinear upsample: out = R @ S @ R^T (bf16 inputs) ----
            # Single K=128 contraction: partitions 32j+t (t<8) carry S rows;
            # all other partitions are zeros (memset) x zero rtp4 rows.
            psu = ppool.tile([128, WA], f32, tag="psA0")
            nc.tensor.matmul(psu[:32, :OUT], trp[:], rtp4_sb[:],
                             start=True, stop=True)
            u_sb = fpool.tile([32, OUT], bf16, tag="u")
            nc.vector.tensor_copy(u_sb[:], psu[:32, :OUT])
            # all 4 output blocks in flight at once: 4 MMs into 4 free PSUM
            # tiles, copies alternate DVE/ACT, DMAs alternate queues
            dmaq = [nc.sync, nc.scalar, nc.sync, nc.scalar]
            po4 = [ppool.tile([128, WA], f32, tag=f"psA{1 - i % 2}",
                              name=f"po{i}") if i < 2 else
                   ppool.tile([128, WD], f32, tag=f"psB{i % 2}",
                              name=f"po{i}")
                   for i in range(4)]
            for i in range(4):
                nc.tensor.matmul(po4[i][:, :OUT],
                                 u_sb[:, i * 128:(i + 1) * 128],
                                 rt_sb[:], start=True, stop=True)
            for i in range(4):
                osb = fpool.tile([128, OUT], f32, tag=f"osb{i}",
                                 name=f"osb{i}")
                if i % 2 == 0:
                    nc.vector.tensor_copy(osb[:], po4[i][:, :OUT])
                else:
                    nc.scalar.copy(osb[:], po4[i][:, :OUT])
                dmaq[i].dma_start(out_d[i * 128:(i + 1) * 128, :], osb[:])

    nc.compile()
    return nc


def _resize_matrix(n_in: int, n_out: int) -> np.ndarray:
    """Bilinear (half-pixel, edge-clamped) interpolation matrix [n_out, n_in].
    Matches jax.image.resize(method='bilinear') for upsampling."""
    R = np.zeros((n_out, n_in), dtype=np.float64)
    scale = n_in / n_out
    for i in range(n_out):
        src = (i + 0.5) * scale - 0.5
        a0 = int(np.floor(src))
        w = src - a0
        a0c = min(max(a0, 0), n_in - 1)
        a1c = min(max(a0 + 1, 0), n_in - 1)
        R[i, a0c] += 1.0 - w
        R[i, a1c] += w
    return R.astype(np.float32)


def _prep_inputs(embeddings: np.ndarray, bank: np.ndarray):
    """Host-side layout prep. Returns per-core input maps."""
    import ml_dtypes
    f = np.float32
    f8 = ml_dtypes.float8_e4m3fn
    emb = np.asarray(embeddings, dtype=f)
    bank = np.asarray(bank, dtype=f)

    # queries: [B, E, HL, WL] -> qT [B, E, Q] (E-major for the stationary side)
    qT = emb.reshape(B, E, Q)
    q2 = np.einsum("beq,beq->bq", qT, qT)               # [B, Q]
    qk8_full = (2.0 * qT).astype(f8)                    # [B, E, Q]
    qk8_full[:, E - 1, :] = f8(C_AUG)                   # merged b2 row
    qk8_all = np.ascontiguousarray(
        qk8_full.reshape(B, KC, 128, Q).transpose(0, 2, 1, 3)
    )                                                   # [B, 128, KC, Q]
    qb_all = ((q2 + 768.0) / 9.0).reshape(B, QT, 128).transpose(0, 2, 1)

    b2 = np.einsum("ne,ne->n", bank, bank)
    bankP = np.zeros((E, NPAD), dtype=f)
    bankP[:, :N_BANK] = bank.T
    bank8f = bankP.astype(f8)
    bank8f[E - 1, :N_BANK] = (-(b2 - 768.0) / C_AUG).astype(f8)
    bank8f[E - 1, N_BANK:] = f8(-448.0)                 # pad: r = -7168
    bank8 = np.ascontiguousarray(
        bank8f.reshape(KC, 128, NG, GW).transpose(2, 0, 1, 3)
    )                                                   # [NG, KC, 128, GW]

    bh = ml_dtypes.bfloat16
    rt = np.ascontiguousarray(_resize_matrix(HL, OUT).T.astype(bh))  # [32, 512]
    rtp4 = np.zeros((128, OUT), dtype=bh)
    for j in range(4):
        for t in range(8):
            rtp4[32 * j + t] = rt[4 * t + j]

    in_maps = [
        {
            "qk8": np.ascontiguousarray(qk8_all[b]),
            "qb": np.ascontiguousarray(qb_all[b].astype(f)),
            "bank8": bank8,
            "rt": rt,
            "rtp4": rtp4,
        }
        for b in range(B)
    ]
    return in_maps


def kernel(embeddings, bank, out_size, _trace=False, _trace_kwargs=None):
    from concourse import bass_utils

    assert int(out_size) == OUT
    if "nc" not in _CACHE:
        _CACHE["nc"] = _build_nc()
    nc = _CACHE["nc"]

    in_maps = _prep_inputs(np.asarray(embeddings), np.asarray(bank))
    res = bass_utils.run_bass_kernel_spmd(
        nc, in_maps, core_ids=list(range(B)), trace=_trace,
        **(_trace_kwargs or {}),
    )
    _CACHE["last_results"] = res
    out = np.stack([res.results[b]["out"] for b in range(B)])
    return out.reshape(B, 1, OUT, OUT).astype(np.float32)
